# revision 13
# baseline (speedup 1.0000x reference)
"""Causal self-attention (GQA + qk RMS-norm + RoPE + q-gain) TRN2 Bass kernel.

Sharding: 8 cores = 2 batches x 4 kv-groups. Core c -> (b = c // 4, g = c % 4):
q heads 4g..4g+3, kv head g. Each core computes a partial projection output
(full [S, D]); host sums the 4 partials per batch.

Per-core program (SPMD, one BIR for all cores):
  inputs:
    xT    [2048, 2048] f32r  x[b].T            (c-major: xT[c, t])
    wq    [2048, 512]  f32r  wq_g.T            ([c, o], o = 4 heads x 128)
    wkv   [2048, 256]  f32r  [wk_g.T | wv_g.T] ([c, 128+128])
    wproj [512, 2048]  fp8e4 wproj[:, 512g:512(g+1)].T * 32  ([o, Dc])
    c2    [2048, 128]  fp16  [cos | cos]  per (t, s, d2)
    s2    [2048, 128]  fp16  [sin | -sin]
    gains [128, 4]     f32   gain[h]/sqrt(128), replicated over partitions
    keepw [128, 896]   fp8e5 staircase: keepw[j, u] = 1.0 if (u - 384) >= j
    ident [128, 128]   fp16  I (for PE transposes)
    ones2 [128, 2]     fp8e5 ones (DoubleRow denominator matmul lhsT)
    onesr [1, 128]     fp16  ones (broadcast matmul lhsT)
  output:
    out   [2048, 2048] fp16  partial projection output (natural [t, Dc]),
                             scaled by 1/32 on device (wproj prescale)

PE work per core (cycles ~= moving rows; fp8 DoubleRow = 0.5 c/row):
  QKV proj f32r 196K + scores fp16 82K + AV fp8-DR 20K + den fp8-DR 20K
  + bc 8K + transposes fp16 10K + out-proj fp8-DR 33K ~= 370K cycles.
"""
import sys

sys.path.insert(0, "/opt/trn_rl_repo")

from contextlib import ExitStack

import numpy as np
import ml_dtypes

import concourse.bacc as bacc
import concourse.tile as tile
import concourse.mybir as mybir

F32 = mybir.dt.float32
F32R = mybir.dt.float32r
FP16 = mybir.dt.float16
FP8E4 = mybir.dt.float8e4
FP8E5 = mybir.dt.float8e5
E4NP = ml_dtypes.float8_e4m3
E5NP = ml_dtypes.float8_e5m2

S = 2048
D = 2048
HD = 128
NH_CORE = 4  # q heads per core
ROPE_BASE = 10000.0
EPS = 1.1920929e-07
NT = S // 128  # 16 t-tiles
NG = 4  # groups of 4 t-tiles
WPSCALE = 32.0
ALU = mybir.AluOpType
AF = mybir.ActivationFunctionType
DR = mybir.MatmulPerfMode.DoubleRow


def build_program(num_devices=8, phases="BCD", fp8av=True, fp8proj=False,
                  repeat=1):
    nc = bacc.Bacc("TRN2", target_bir_lowering=False, debug=False,
                   num_devices=num_devices)

    exdt = FP8E5 if fp8av else FP16
    ydt = FP8E4 if fp8proj else FP16

    xT = nc.dram_tensor("xT", (D, S), F32R, kind="ExternalInput").ap()
    wq = nc.dram_tensor("wq", (D, 512), F32R, kind="ExternalInput").ap()
    wkv = nc.dram_tensor("wkv", (D, 256), F32R, kind="ExternalInput").ap()
    wproj = nc.dram_tensor("wproj", (512, D), ydt, kind="ExternalInput").ap()
    c2d = nc.dram_tensor("c2", (S, 128), FP16, kind="ExternalInput").ap()
    s2d = nc.dram_tensor("s2", (S, 128), FP16, kind="ExternalInput").ap()
    gainsd = nc.dram_tensor("gains", (128, 4), F32, kind="ExternalInput").ap()
    keepwd = nc.dram_tensor("keepw", (128, 896), exdt, kind="ExternalInput").ap()
    keepw16d = nc.dram_tensor("keepw16", (128, 896), FP16, kind="ExternalInput").ap()
    identd = nc.dram_tensor("ident", (128, 128), FP16, kind="ExternalInput").ap()
    ones2d = nc.dram_tensor("ones2", (128, 256), exdt, kind="ExternalInput").ap()
    onesrd = nc.dram_tensor("onesr", (1, 128), FP16, kind="ExternalInput").ap()
    outd = nc.dram_tensor("out", (S, D), FP16, kind="ExternalOutput").ap()

    with tile.TileContext(nc) as tc:
      for rep in range(repeat):
        P = f"r{rep}_"
        ctx = ExitStack()
        # ---------------- persistent pools ----------------
        sbc = ctx.enter_context(tc.tile_pool(name=P + "consts", bufs=1))
        qtp = ctx.enter_context(tc.tile_pool(name=P + "qtp", bufs=16))

        # ---------------- constants / weights resident ----------------
        wq_sb = sbc.tile([128, 16 * 512], F32R, tag="wq")
        wkv_sb = sbc.tile([128, 16 * 256], F32R, tag="wkv")
        for k in range(0, 16, 2):
            nc.gpsimd.dma_start(
                wq_sb[:, k * 512:(k + 2) * 512].rearrange(
                    "p (k o) -> p k o", k=2),
                wq[k * 128:(k + 2) * 128, :].rearrange(
                    "(k p) o -> p k o", p=128),
            )
            nc.gpsimd.dma_start(
                wkv_sb[:, k * 256:(k + 2) * 256].rearrange(
                    "p (k o) -> p k o", k=2),
                wkv[k * 128:(k + 2) * 128, :].rearrange(
                    "(k p) o -> p k o", p=128),
            )
        c2_sb = sbc.tile([128, 16 * 128], FP16, tag="c2")
        nc.gpsimd.dma_start(
            c2_sb[:].rearrange("p (i d) -> p i d", i=16),
            c2d.rearrange("(i p) d -> p i d", p=128),
        )
        s2_sb = sbc.tile([128, 16 * 128], FP16, tag="s2")
        nc.gpsimd.dma_start(
            s2_sb[:].rearrange("p (i d) -> p i d", i=16),
            s2d.rearrange("(i p) d -> p i d", p=128),
        )
        gains_sb = sbc.tile([128, 4], F32, tag="gains")
        nc.gpsimd.dma_start(gains_sb[:], gainsd)
        keepw_sb = sbc.tile([128, 896], exdt, tag="keepw")
        nc.gpsimd.dma_start(keepw_sb[:], keepwd)
        keepw16_sb = sbc.tile([128, 896], FP16, tag="keepw16")
        nc.gpsimd.dma_start(keepw16_sb[:], keepw16d)
        ident_sb = sbc.tile([128, 128], FP16, tag="ident")
        nc.gpsimd.dma_start(ident_sb[:], identd)
        ones2_sb = sbc.tile([128, 256], exdt, tag="ones2")
        nc.gpsimd.dma_start(ones2_sb[:], ones2d)
        onesr_sb = sbc.tile([1, 128], FP16, tag="onesr")
        nc.gpsimd.dma_start(onesr_sb[:], onesrd)

        negb_sb = sbc.tile([128, 1], F32, tag="negb")
        nc.vector.memset(negb_sb[:], -1.0)
        ones16_sb = sbc.tile([128, 1], FP16, tag="ones16")
        nc.vector.memset(ones16_sb[:], 1.0)
        v16_sb = sbc.tile([128, 4 * 128], FP16, tag="v16")
        kT_sb = sbc.tile([128, 16 * 128], FP16, tag="kT")   # k final, transposed
        v_sb = sbc.tile([128, 16 * 128], FP8E4 if fp8av else FP16, tag="v")

        qT = {}    # (h, g) -> [128 d, 512 t] fp16 tile
        ytile = {}  # (hp, qc) -> [128 d, 2*512 t] pair tile (h = 2hp, 2hp+1)

        # ================ phase B: projections + norm + rope + transpose ====
        ctxB = ExitStack()
        io2k = ctxB.enter_context(tc.tile_pool(name=P + "io2k", bufs=17))
        work = ctxB.enter_context(tc.tile_pool(name=P + "work", bufs=1))
        qfp = ctxB.enter_context(tc.tile_pool(name=P + "qfp", bufs=1))
        smp = ctxB.enter_context(tc.tile_pool(name=P + "smp", bufs=4))
        psQ = ctxB.enter_context(tc.tile_pool(name=P + "psQ", bufs=2, space="PSUM"))
        psKV = ctxB.enter_context(tc.tile_pool(name=P + "psKV", bufs=2, space="PSUM"))
        psTR = ctxB.enter_context(tc.tile_pool(name=P + "psTR", bufs=2, space="PSUM"))

        for g in range(NG):
            xts = []
            for k in range(16):
                xt = io2k.tile([128, 512], F32R, tag="io", name=f"{P}xt_{g}_{k}")
                nc.sync.dma_start(
                    xt[:], xT[k * 128:(k + 1) * 128, g * 512:(g + 1) * 512]
                )
                xts.append(xt)
            xts = [t[:] for t in xts]

            qn = work.tile([128, 2048], FP16, tag="qn", name=f"{P}qn_{g}")
            kn = work.tile([128, 512], FP16, tag="kn", name=f"{P}kn_{g}", bufs=2)
            for tt in range(4):
                i = g * 4 + tt
                psq = psQ.tile([128, 512], F32, tag="pQ", name=f"{P}psq_{i}")
                for k in range(16):
                    nc.tensor.matmul(
                        psq[:],
                        xts[k][:, tt * 128:(tt + 1) * 128],
                        wq_sb[:, k * 512:(k + 1) * 512],
                        start=(k == 0), stop=(k == 15),
                    )
                pskv = psKV.tile([128, 256], F32, tag="pKV", name=f"{P}pskv_{i}")
                for k in range(16):
                    nc.tensor.matmul(
                        pskv[:],
                        xts[k][:, tt * 128:(tt + 1) * 128],
                        wkv_sb[:, k * 256:(k + 1) * 256],
                        start=(k == 0), stop=(k == 15),
                    )

                # evacuate raw to fp16 (2-byte => 2x DVE rate downstream)
                qraw = smp.tile([128, 512], FP16, tag="qraw",
                                name=f"{P}qraw_{i}", bufs=2)
                nc.vector.tensor_copy(qraw[:], psq[:])
                kvraw = smp.tile([128, 256], FP16, tag="kvraw",
                                 name=f"{P}kvraw_{i}", bufs=2)
                nc.vector.tensor_copy(kvraw[:], pskv[:])
                ms = smp.tile([128, 5], F32, tag="ms", name=f"{P}ms_{i}")
                for h in range(NH_CORE):
                    nc.vector.scalar_tensor_tensor(
                        out=qn[:, tt * 512 + h * 128: tt * 512 + (h + 1) * 128],
                        in0=qraw[:, h * 128:(h + 1) * 128],
                        scalar=1.0,
                        in1=qraw[:, h * 128:(h + 1) * 128],
                        op0=ALU.mult, op1=ALU.mult,
                        accum_out=ms[:, h:h + 1],
                    )
                nc.vector.scalar_tensor_tensor(
                    out=kn[:, tt * 128:(tt + 1) * 128],
                    in0=kvraw[:, 0:128], scalar=1.0, in1=kvraw[:, 0:128],
                    op0=ALU.mult, op1=ALU.mult,
                    accum_out=ms[:, 4:5],
                )
                msx = smp.tile([128, 5], F32, tag="msx", name=f"{P}msx_{i}")
                nc.vector.tensor_scalar(msx[:], ms[:], 1.0 / HD, EPS,
                                        op0=ALU.mult, op1=ALU.add)
                u = smp.tile([128, 5], F32, tag="u", name=f"{P}u_{i}")
                usc = smp.tile([128, 5], F32, tag="usc", name=f"{P}usc_{i}")
                nc.vector.reciprocal_approx_accurate(out=u[:], in_=msx[:],
                                                     scratch=usc[:])
                rin = smp.tile([128, 5], F32, tag="rin", name=f"{P}rin_{i}")
                nc.scalar.activation(rin[:], u[:], AF.Sqrt)
                ring = smp.tile([128, 4], F32, tag="ring", name=f"{P}ring_{i}")
                nc.vector.tensor_mul(ring[:], rin[:, 0:4], gains_sb[:])

                # scale into qn / kn, copy v (fp8 cast happens on write)
                for h in range(NH_CORE):
                    nc.vector.tensor_scalar_mul(
                        qn[:, tt * 512 + h * 128: tt * 512 + (h + 1) * 128],
                        qraw[:, h * 128:(h + 1) * 128],
                        ring[:, h:h + 1],
                    )
                nc.vector.tensor_scalar_mul(
                    kn[:, tt * 128:(tt + 1) * 128],
                    kvraw[:, 0:128], rin[:, 4:5],
                )
                nc.scalar.copy(
                    v_sb[:, i * 128:(i + 1) * 128], kvraw[:, 128:256],
                )
                if i < 4:
                    nc.scalar.copy(
                        v16_sb[:, i * 128:(i + 1) * 128], kvraw[:, 128:256],
                    )

            # ---- rope on q group: [tt, h, s, d2] layout ----
            qn5 = qn[:].rearrange("p (tt h s d) -> p tt h s d", tt=4, h=4, s=2)
            c2g = (
                c2_sb[:].rearrange("p (i one s d) -> p i one s d",
                                   i=16, one=1, s=2)
                [:, g * 4:(g + 1) * 4]
                .broadcast_to((128, 4, 4, 2, 64))
            )
            s2g = (
                s2_sb[:].rearrange("p (i one s d) -> p i one s d",
                                   i=16, one=1, s=2)
                [:, g * 4:(g + 1) * 4]
                .broadcast_to((128, 4, 4, 2, 64))
            )
            t1 = work.tile([128, 2048], FP16, tag="rt", name=f"{P}t1_{g}")
            qf = qfp.tile([128, 2048], FP16, tag="qf", name=f"{P}qf_{g}")
            t15 = t1[:].rearrange("p (tt h s d) -> p tt h s d", tt=4, h=4, s=2)
            qf5 = qf[:].rearrange("p (tt h s d) -> p tt h s d", tt=4, h=4, s=2)
            nc.vector.tensor_mul(t15[:, :, :, 0:1, :], qn5[:, :, :, 1:2, :],
                                 s2g[:, :, :, 0:1, :])
            nc.vector.tensor_mul(t15[:, :, :, 1:2, :], qn5[:, :, :, 0:1, :],
                                 s2g[:, :, :, 1:2, :])
            nc.vector.tensor_mul(qf5, qn5, c2g)
            nc.vector.tensor_add(qf[:], qf[:], t1[:])

            # ---- rope on k group: [i(4), s, d2] layout ----
            kn4 = kn[:].rearrange("p (i s d) -> p i s d", i=4, s=2)
            kc2 = c2_sb[:, g * 512:(g + 1) * 512].rearrange(
                "p (i s d) -> p i s d", i=4, s=2)
            ks2 = s2_sb[:, g * 512:(g + 1) * 512].rearrange(
                "p (i s d) -> p i s d", i=4, s=2)
            kt1 = work.tile([128, 512], FP16, tag="kt", name=f"{P}kt1_{g}", bufs=2)
            kf = work.tile([128, 512], FP16, tag="kf", name=f"{P}kf_{g}", bufs=2)
            kt14 = kt1[:].rearrange("p (i s d) -> p i s d", i=4, s=2)
            kf4 = kf[:].rearrange("p (i s d) -> p i s d", i=4, s=2)
            nc.vector.tensor_mul(kt14[:, :, 0:1, :], kn4[:, :, 1:2, :],
                                 ks2[:, :, 0:1, :])
            nc.vector.tensor_mul(kt14[:, :, 1:2, :], kn4[:, :, 0:1, :],
                                 ks2[:, :, 1:2, :])
            nc.vector.tensor_mul(kf4, kn4, kc2)
            nc.vector.tensor_add(kf[:], kf[:], kt1[:])

            # ---- transposes: q (h, g) -> qT, k -> kT_sb ----
            for h in range(NH_CORE):
                trp = psTR.tile([128, 512], FP16, tag="pTR", name=f"{P}trq_{g}_{h}")
                for tt in range(4):
                    nc.tensor.transpose(
                        trp[:, tt * 128:(tt + 1) * 128],
                        qf[:, tt * 512 + h * 128: tt * 512 + (h + 1) * 128],
                        ident_sb[:],
                    )
                qt = qtp.tile([128, 512], FP16, tag="qT", name=f"{P}qT_{g}_{h}")
                nc.scalar.copy(qt[:], trp[:])
                qT[(h, g)] = qt
            trk = psTR.tile([128, 512], FP16, tag="pTR", name=f"{P}trk_{g}")
            for tt in range(4):
                nc.tensor.transpose(
                    trk[:, tt * 128:(tt + 1) * 128],
                    kf[:, tt * 128:(tt + 1) * 128],
                    ident_sb[:],
                )
            nc.scalar.copy(kT_sb[:, g * 512:(g + 1) * 512], trk[:])
        ctxB.close()

        # ================ phase C: attention (+ interleaved phase D) ========
        ctxC = ExitStack()
        expp = ctxC.enter_context(tc.tile_pool(name=P + "expp", bufs=3))
        smc = ctxC.enter_context(tc.tile_pool(name=P + "smc", bufs=2))
        ytp = ctxC.enter_context(tc.tile_pool(name=P + "ytp", bufs=8))
        wpp = ctxC.enter_context(tc.tile_pool(name=P + "wpp", bufs=8))
        outp = ctxC.enter_context(tc.tile_pool(name=P + "outp", bufs=2))
        psSC = ctxC.enter_context(tc.tile_pool(name=P + "psSC", bufs=3, space="PSUM"))
        psYT = ctxC.enter_context(tc.tile_pool(name=P + "psYT", bufs=2, space="PSUM"))
        psDB = ctxC.enter_context(tc.tile_pool(name=P + "psDB", bufs=1, space="PSUM"))
        psFP = ctxC.enter_context(tc.tile_pool(name=P + "psFP", bufs=2, space="PSUM"))

        wp = {}
        if "D" in phases:
            for hp in range(2):
                for dc in range(4):
                    w = wpp.tile([128, 2 * 512], ydt, tag="wp",
                                 name=f"{P}wp_{hp}_{dc}")
                    nc.gpsimd.dma_start(
                        w[:].rearrange("p (two o) -> p two o", two=2),
                        wproj[hp * 256:(hp + 1) * 256,
                              dc * 512:(dc + 1) * 512].rearrange(
                            "(two p) o -> p two o", p=128),
                    )
                    wp[(hp, dc)] = w

        for qc in range(4 if "C" in phases else 0):
            npair = 2 * qc + 2
            qfp8 = fp8av and qc > 0  # early rows: no averaging, keep fp16
            qexdt = exdt if qfp8 else FP16
            qkeep = keepw_sb if qfp8 else keepw16_sb
            for h in range(NH_CORE):
                yt_ps = psYT.tile([128, 512], F32, tag="pYT", name=f"{P}yt_{qc}_{h}")
                den_ps = psDB.tile([128, 512] if qfp8 else [1, 512], F32,
                                   tag="pDB", name=f"{P}den_{qc}_{h}")
                for jp in range(npair):
                    j0, j1 = 2 * jp, 2 * jp + 1
                    ex2 = expp.tile([128, 1024], qexdt, tag="exp",
                                    name=f"{P}ex_{qc}_{h}_{jp}")
                    for half, j in ((0, j0), (1, j1)):
                        sc = psSC.tile([128, 512], F32, tag="pSC",
                                       name=f"{P}sc_{qc}_{h}_{j}")
                        nc.tensor.matmul(
                            sc[:],
                            kT_sb[:, j * 128:(j + 1) * 128],
                            qT[(h, qc)][:],
                            start=True, stop=True,
                        )
                        exh = ex2[:, half * 512:(half + 1) * 512]
                        nc.scalar.activation(exh, sc[:], AF.Exp,
                                             bias=negb_sb[:])
                        if j >= 4 * qc:  # diagonal tile: causal mask
                            dlt = 128 * j - 512 * qc
                            nc.vector.tensor_mul(
                                exh, exh,
                                qkeep[:, 384 - dlt: 896 - dlt],
                            )
                    ex2r = ex2[:].rearrange("p (two t) -> p two t", two=2)
                    if qfp8:
                        nc.tensor.matmul(
                            den_ps[:],
                            ones2_sb[:].rearrange("p (two o) -> p two o",
                                                  two=2),
                            ex2r, start=(jp == 0), stop=(jp == npair - 1),
                            perf_mode=DR,
                        )  # den broadcast to all 128 rows (ones weights)
                        nc.tensor.matmul(
                            yt_ps[:],
                            v_sb[:, j0 * 128:(j1 + 1) * 128].rearrange(
                                "p (two d) -> p two d", two=2),
                            ex2r, start=(jp == 0), stop=(jp == npair - 1),
                            perf_mode=DR,
                        )
                    else:
                        for half, j in ((0, j0), (1, j1)):
                            exh = ex2[:, half * 512:(half + 1) * 512]
                            nc.tensor.matmul(
                                den_ps[:], ones16_sb[:], exh,
                                start=(j == 0), stop=(j == 2 * npair - 1),
                            )
                            nc.tensor.matmul(
                                yt_ps[:],
                                v16_sb[:, j * 128:(j + 1) * 128], exh,
                                start=(j == 0), stop=(j == 2 * npair - 1),
                            )
                bc_sb = smc.tile([128, 512], F32, tag="bcs",
                                 name=f"{P}bcs_{qc}_{h}")
                if qfp8:
                    # den already broadcast across partitions by ones lhsT
                    nc.vector.reciprocal_approx_fast(out=bc_sb[:],
                                                     in_=den_ps[:])
                else:
                    rinv = smc.tile([1, 512], F32, tag="rinv",
                                    name=f"{P}rinv_{qc}_{h}")
                    nc.vector.reciprocal_approx_fast(out=rinv[:],
                                                     in_=den_ps[:])
                    rinv16 = smc.tile([1, 512], FP16, tag="rinv16",
                                      name=f"{P}rinv16_{qc}_{h}")
                    nc.vector.tensor_copy(rinv16[:], rinv[:])
                    bc_ps = psDB.tile([128, 512], F32, tag="pDB",
                                      name=f"{P}bc_{qc}_{h}")
                    nc.tensor.matmul(bc_ps[:], onesr_sb[:], rinv16[:],
                                     start=True, stop=True)
                    nc.vector.tensor_copy(bc_sb[:], bc_ps[:])
                hp, hh = h // 2, h % 2
                if (hp, qc) not in ytile:
                    ytile[(hp, qc)] = ytp.tile(
                        [128, 2 * 512], ydt, tag="yt", name=f"{P}ytsb_{qc}_{hp}")
                nc.vector.tensor_mul(
                    ytile[(hp, qc)][:, hh * 512:(hh + 1) * 512],
                    yt_ps[:], bc_sb[:])

            # ---- phase D chunk for this qc: output projection ----
            if "D" in phases:
                for tt in range(4):
                    i = qc * 4 + tt
                    for dc2 in range(2):
                        ob = outp.tile([128, 1024], FP16, tag="ob",
                                       name=f"{P}ob_{i}_{dc2}")
                        for half in range(2):
                            dc = dc2 * 2 + half
                            fp = psFP.tile([128, 512], F32, tag="pFP",
                                           name=f"{P}fp_{i}_{dc}")
                            for hp in range(2):
                                yp = ytile[(hp, qc)][:].rearrange(
                                    "p (two t) -> p two t", two=2)
                                if fp8proj:
                                    nc.tensor.matmul(
                                        fp[:],
                                        yp[:, :, tt * 128:(tt + 1) * 128],
                                        wp[(hp, dc)][:].rearrange(
                                            "p (two o) -> p two o", two=2),
                                        start=(hp == 0), stop=(hp == 1),
                                        perf_mode=DR,
                                    )
                                else:
                                    for hh in range(2):
                                        h = 2 * hp + hh
                                        nc.tensor.matmul(
                                            fp[:],
                                            ytile[(hp, qc)][
                                                :, hh * 512 + tt * 128:
                                                hh * 512 + (tt + 1) * 128],
                                            wp[(hp, dc)][
                                                :, hh * 512:(hh + 1) * 512],
                                            start=(h == 0), stop=(h == 3),
                                        )
                            obh = ob[:, half * 512:(half + 1) * 512]
                            if (i + dc) % 2 == 0:
                                nc.vector.tensor_scalar_mul(
                                    obh, fp[:], 1.0 / WPSCALE)
                            else:
                                nc.scalar.mul(obh, fp[:], 1.0 / WPSCALE)
                        nc.gpsimd.dma_start(
                            outd[i * 128:(i + 1) * 128,
                                 dc2 * 1024:(dc2 + 1) * 1024],
                            ob[:],
                        )
        ctxC.close()
        ctx.close()

    nc.compile()
    return nc


# ---------------- host-side helpers ----------------

def rope_tables():
    inv_freq = 1.0 / (ROPE_BASE ** (np.arange(0, HD, 2, dtype=np.float32) / HD))
    t = np.arange(S, dtype=np.float32)
    fr = np.outer(t, inv_freq)
    cos = np.cos(fr).astype(np.float16)
    sin = np.sin(fr).astype(np.float16)
    c2 = np.concatenate([cos, cos], axis=1)
    s2 = np.concatenate([sin, -sin], axis=1)
    return c2, s2


def make_consts(fp8av=True):
    exnp = E5NP if fp8av else np.float16
    c2, s2 = rope_tables()
    j = np.arange(128)[:, None]
    u = np.arange(896)[None, :]
    keepw = ((u - 384) >= j).astype(exnp)
    keepw16 = ((u - 384) >= j).astype(np.float16)
    ident = np.eye(128, dtype=np.float16)
    ones2 = np.ones((128, 256), exnp)
    onesr = np.ones((1, 128), np.float16)
    return dict(c2=c2, s2=s2, keepw=keepw, keepw16=keepw16, ident=ident,
                ones2=ones2, onesr=onesr)


def make_core_inputs(x, wq, wk, wv, wproj, q_gain, core, consts=None,
                     fp8av=True, fp8proj=False):
    """x: [B, S, D] f32; returns in_map for `core` (0..7)."""
    if consts is None:
        consts = make_consts(fp8av)
    ynp = E4NP if fp8proj else np.float16
    b, g = core // 4, core % 4
    xTc = np.ascontiguousarray(x[b].T)                       # [D, S]
    wqc = np.ascontiguousarray(wq[g * 512:(g + 1) * 512].T)  # [D, 512]
    wkc = wk[g * 128:(g + 1) * 128].T                        # [D, 128]
    wvc = wv[g * 128:(g + 1) * 128].T
    wkvc = np.ascontiguousarray(np.concatenate([wkc, wvc], axis=1))
    wpc = np.ascontiguousarray(
        (wproj[:, g * 512:(g + 1) * 512].T * WPSCALE).astype(ynp))  # [512, D]
    gains = np.broadcast_to(
        (q_gain[g * 4:(g + 1) * 4] / np.sqrt(HD)).astype(np.float32)[None, :],
        (128, 4),
    ).copy()
    return dict(
        xT=xTc, wq=wqc, wkv=wkvc, wproj=wpc,
        c2=consts["c2"], s2=consts["s2"], gains=gains,
        keepw=consts["keepw"], keepw16=consts["keepw16"],
        ident=consts["ident"],
        ones2=consts["ones2"], onesr=consts["onesr"],
    )


# ---------------- public entry point ----------------

_PROGRAM = None


def _get_program():
    global _PROGRAM
    if _PROGRAM is None:
        _PROGRAM = build_program()
    return _PROGRAM


def kernel(x, wq, wk, wv, wproj, q_gain):
    """Causal self-attention forward. Full inputs in, full output out.

    Shards across 8 NeuronCores as 2 batches x 4 kv-head groups
    (tensor-parallel over heads); each core produces a partial output
    projection; partials are summed per batch on the host (the unshard
    step for input-dim-sharded wproj).
    """
    from concourse.bass_utils import run_bass_kernel_spmd

    x = np.ascontiguousarray(np.asarray(x, dtype=np.float32))
    wq = np.ascontiguousarray(np.asarray(wq, dtype=np.float32))
    wk = np.ascontiguousarray(np.asarray(wk, dtype=np.float32))
    wv = np.ascontiguousarray(np.asarray(wv, dtype=np.float32))
    wproj = np.ascontiguousarray(np.asarray(wproj, dtype=np.float32))
    q_gain = np.asarray(q_gain, dtype=np.float32)

    nc = _get_program()
    consts = make_consts()
    in_maps = [make_core_inputs(x, wq, wk, wv, wproj, q_gain, c, consts)
               for c in range(8)]
    res = run_bass_kernel_spmd(nc, in_maps, core_ids=list(range(8)))
    parts = [r["out"].astype(np.float32) for r in res.results]
    y = np.stack([
        parts[0] + parts[1] + parts[2] + parts[3],
        parts[4] + parts[5] + parts[6] + parts[7],
    ]).astype(np.float32)
    return y
